# revision 18
# baseline (speedup 1.0000x reference)
"""Trainium2 Bass kernel for nn_CFAggregator (GNN message passing).

Strategy (B-sharded data parallel over 8 cores, no collectives):
  - Host: all indexed loads are pre-staged per core. The edge feature rows
    (agg_table[unique_ids[col_idx]]) are laid out in PE-ready chunk-slot
    order as an fp8 [128, nchk*128] tensor (partition = slot-in-chunk), so
    the on-device "gather" is one contiguous full-bandwidth DMA. The self
    features are staged pre-transposed (pair_T [feat, node]), removing the
    on-device PE transposes. All synthesized constants (ones, 1/DOUT, the
    mu_w vectors pre-folded through Wv_agg) are materialized into a single
    packed fp16 tensor. The dedup'd edge weights (mask .set() + 1/cnt) ride
    in a block one-hot A matrix (fp16) mapping slots to dest columns.
  - Device: plain DMAs (HWDGE spread across SP/ACT/DVE queues, tab8 split
    into pieces to pipeline with PE), PE accumulates G_chunk^T @ A_chunk
    into four 256-column PSUM quarters. Each quarter runs an independent
    chain (Wv matmul, stats matmuls with 0-stride broadcast self fold-in,
    1/sqrt via exp(-0.5 ln) on a single forced exp+ln activation table,
    softmax over MC, highway attention mix, ELU) with ops globally ordered
    by estimated start time and balanced across ACT/DVE/Pool; per-quarter
    output DMAs overlap the remaining chains.
The host only performs index math, dtype conversion, and row restaging;
all arithmetic on feature values happens on-device.
"""

import numpy as np
import ml_dtypes

import concourse.bass as bass
import concourse.bacc as bacc
import concourse.tile as tile
from concourse import mybir
from concourse.bass_utils import run_bass_kernel_spmd

F32 = mybir.dt.float32
F16 = mybir.dt.float16
F8 = mybir.dt.float8e4
I32 = mybir.dt.int32
AF = mybir.ActivationFunctionType
OP = mybir.AluOpType
NPF16 = np.float16
NPF8 = ml_dtypes.float8_e4m3fn

# problem dims (hardcoded per contract)
B, MC, U, N, DIN, DOUT, E = 2048, 4, 20000, 100000, 128, 128, 65536
RES_RATE = 0.9
NCORES = 8
BC = B // NCORES          # 256 nodes per core
DEST = BC * MC            # 1024 destination columns per core
P = 128

# consts tile slots (each [128, 128] fp16)
(S_WK, S_WQ, S_WVA, S_WVF, S_ONES, S_ONESC, S_MUHI, S_MULO, S_ID) = range(9)
NSLOT = 9
CW = NSLOT * 128          # consts width
PKW = CW + 512            # packed: consts | pair_T
# per-quarter engine strings (index q): 'a'=ACT, 'd'=DVE, 'g'=Pool
ENG = dict(nraw='dada', sqn='aaaa', tmul='dddd', nw='dd', pre='gg',
           rp='gg', stt='dd', soff=1.0)


# --------------------------------------------------------------------------
# host-side preprocessing (index math + row restaging only)
# --------------------------------------------------------------------------

def preprocess(inputs):
    """Build per-core staged tensors + seg plan. Returns (plan, percore)."""
    nodes = np.asarray(inputs["nodes"]).astype(np.int64)
    unique_ids = np.asarray(inputs["unique_ids"]).astype(np.int64)
    row_idx = np.asarray(inputs["row_idx"]).astype(np.int64)
    layer_idx = np.asarray(inputs["layer_idx"]).astype(np.int64)
    col_idx = np.asarray(inputs["col_idx"]).astype(np.int64)

    eff = unique_ids[col_idx]                       # table row per edge
    # dedup (b, layer, col) triples: .set() counts duplicates once
    key = (row_idx * MC + layer_idx) * U + col_idx
    uniq_keys, first_pos = np.unique(key, return_index=True)
    keep = np.zeros(E, bool)
    keep[first_pos] = True
    grp_of_uniq = uniq_keys // U
    cnt = np.bincount(grp_of_uniq, minlength=B * MC)
    grp = row_idx * MC + layer_idx
    w = np.where(keep, 1.0 / np.maximum(cnt[grp], 1), 0.0).astype(np.float32)
    dest_all = (row_idx % BC) * MC + layer_idx      # core-local dest column

    # per-core dest-sorted edge stream
    core_lists = []
    for c in range(NCORES):
        sel = (row_idx >= c * BC) & (row_idx < (c + 1) * BC)
        order = np.argsort(dest_all[sel], kind="stable")
        core_lists.append((eff[sel][order], dest_all[sel][order], w[sel][order]))

    mx = max(len(cl[0]) for cl in core_lists)
    nchk = (mx + 127) // 128
    cap = nchk * 128

    core_streams = []   # (idx, dest, w) padded to cap, quantile-aligned
    for c in range(NCORES):
        idxs, dests, ws = core_lists[c]
        n = len(idxs)
        s_idx = np.full(cap, -1, np.int64)
        s_dst = np.full(cap, -1, np.int64)
        s_w = np.zeros(cap, np.float32)
        bnd = np.round(np.arange(nchk + 1) * n / nchk).astype(np.int64)
        for k in range(nchk):
            e0, e1 = bnd[k], bnd[k + 1]
            s_idx[k * 128:k * 128 + e1 - e0] = idxs[e0:e1]
            s_dst[k * 128:k * 128 + e1 - e0] = dests[e0:e1]
            s_w[k * 128:k * 128 + e1 - e0] = ws[e0:e1]
        core_streams.append((s_idx, s_dst, s_w))

    # per-chunk dest spans = union over cores of real dests
    spans = []
    for k in range(nchk):
        lo, hi = DEST, 0
        for c in range(NCORES):
            d = core_streams[c][1][k * 128:(k + 1) * 128]
            d = d[d >= 0]
            if len(d):
                lo = min(lo, int(d.min()))
                hi = max(hi, int(d.max()) + 1)
        if hi <= lo:
            lo, hi = -1, -1
        spans.append((k, lo, hi))

    # segments: split spans at 256-column quarter-tile boundaries
    segs = []
    acol = 0
    for (k, lo, hi) in spans:
        if lo < 0:
            continue
        for t in range(4):
            b0, b1 = t * 256, (t + 1) * 256
            s0, s1 = max(lo, b0), min(hi, b1)
            if s1 > s0:
                segs.append(dict(chunk=k, tile=t, lo=s0, hi=s1,
                                 acol=acol + (s0 - lo)))
        acol += hi - lo
    aw = max(acol, 1)

    # tab8 DMA pieces: split at quarter-tile boundaries so tile q waits
    # only on piece q (dest-sorted chunks make this near-exact)
    qchunk = []
    for t in range(4):
        ks = [s["chunk"] for s in segs if s["tile"] == t]
        qchunk.append(max(ks) + 1 if ks else None)
    pb = [0]
    for t in range(4):
        nxt = qchunk[t] if qchunk[t] is not None else pb[-1]
        nxt = max(nxt, pb[-1])
        pb.append(min(nxt, nchk))
    pb[-1] = nchk
    pieces = tuple((int(pb[i]), int(pb[i + 1])) for i in range(4)
                   if pb[i + 1] > pb[i])

    plan = dict(nchk=nchk, aw=aw, segs=segs, pieces=pieces)

    agg8 = np.asarray(inputs["agg_table"], np.float32).astype(NPF8)
    agg16 = np.asarray(inputs["agg_table"], np.float32).astype(NPF16)
    ff16 = np.asarray(inputs["ff_table"], np.float32).astype(NPF16)

    span_acol = {}
    ac = 0
    for (k, lo, hi) in spans:
        span_acol[k] = (ac, lo)
        if lo >= 0:
            ac += hi - lo

    percore = []
    for c in range(NCORES):
        s_idx, s_dst, s_w = core_streams[c]
        # staged edge rows, PE layout: [slot-in-chunk (partition), chunk*128+feat]
        tab8 = np.zeros((P, nchk * 128), NPF8)
        rows = agg8[np.maximum(s_idx, 0)]           # [cap, 128]
        rows[s_idx < 0] = 0
        tab8[:] = rows.reshape(nchk, 128, 128).transpose(1, 0, 2).reshape(P, -1)
        # A matrix
        amat = np.zeros((P, aw), NPF16)
        for k in range(nchk):
            a0, lo = span_acol[k]
            if lo < 0:
                continue
            sl = slice(k * 128, (k + 1) * 128)
            real = s_dst[sl] >= 0
            pp = np.nonzero(real)[0]
            amat[pp, a0 + s_dst[sl][pp] - lo] = s_w[sl][pp].astype(NPF16)
        # pair_T staged pre-transposed: [feat, h*128+p] (agg cols 0:256, ff 256:512)
        nd = nodes[c * BC:(c + 1) * BC]
        pairT = np.zeros((P, 512), NPF16)
        pairT[:, 0:256] = agg16[nd].T
        pairT[:, 256:512] = ff16[nd].T
        percore.append(dict(tab8=tab8, amat=amat, pairT=pairT))

    return plan, percore


def make_consts(inputs):
    """[128, CW] fp16 consts block (weights + synthesized constants)."""
    c = np.zeros((P, CW), NPF16)
    c[:, S_WK * 128:(S_WK + 1) * 128] = np.asarray(inputs["Wk"], np.float32)
    c[:, S_WQ * 128:(S_WQ + 1) * 128] = np.asarray(inputs["Wq"], np.float32)
    c[:, S_WVA * 128:(S_WVA + 1) * 128] = np.asarray(inputs["Wv_agg"], np.float32)
    c[:, S_WVF * 128:(S_WVF + 1) * 128] = np.asarray(inputs["Wv_ff"], np.float32)
    c[:, S_ONES * 128:(S_ONES + 1) * 128] = 1.0
    c[:, S_ONESC * 128:(S_ONESC + 1) * 128] = 1.0 / DOUT
    mu = np.asarray(inputs["mu_w"]).astype(np.float32).reshape(2, DOUT).T
    wva = np.asarray(inputs["Wv_agg"], np.float32)
    w_num = (wva @ mu[:, 1]).astype(NPF16)          # neighbor vector
    w_smu = (wva @ mu[:, 0]).astype(NPF16)          # self vector
    c[:, S_MUHI * 128:(S_MUHI + 1) * 128] = w_num[:, None]
    c[:, S_MULO * 128:(S_MULO + 1) * 128] = w_smu[:, None]
    c[:, S_ID * 128:(S_ID + 1) * 128] = np.eye(P, dtype=NPF16)
    return c


# --------------------------------------------------------------------------
# device module
# --------------------------------------------------------------------------

def build_module(plan):
    nchk = plan["nchk"]
    aw = plan["aw"]
    segs = plan["segs"]
    pieces = plan["pieces"]

    nc = bacc.Bacc("TRN2", target_bir_lowering=False, debug=False,
                   num_devices=NCORES)

    import concourse.hw_specs as hw_specs
    orig_tables = hw_specs.get_activation_tables(nc.m.arch)
    tnames = list(orig_tables)
    target = next(n for n in tnames
                  if {AF.Exp, AF.Ln, AF.Copy} <= orig_tables[n])
    target_id = tnames.index(target)

    pk1 = nc.dram_tensor("pk1", [P, PKW], F16, kind="ExternalInput")
    amat = nc.dram_tensor("amat", [P, aw], F16, kind="ExternalInput")
    tab8 = nc.dram_tensor("tab8", [P, nchk * 128], F8, kind="ExternalInput")
    out_t = nc.dram_tensor("out", [2, P, BC], F16, kind="ExternalOutput")

    # ---- DMA timeline model (for scheduler wait hints) ------------------
    # transfer order: amat, piece0, pk1, piece1.. (amat+p0 unblock PE first;
    # pk1 feeds the early-dense block; engines chosen so DGE stays ahead)
    piece_eng = ["gp", "sp", "act", "gp", "sp", "act"]
    dma_items = [("amat", aw * 2, "sp"),
                 ("p0", (pieces[0][1] - pieces[0][0]) * 128, piece_eng[0]),
                 ("pk1", PKW * 2, "act")]
    for i, (k0, k1) in enumerate(pieces[1:], start=1):
        dma_items.append((f"p{i}", (k1 - k0) * 128, piece_eng[i]))
    HWD = dict(sp=625.0, act=632.0)
    SEQ0 = dict(sp=25.0, act=32.0, gp=61.0)
    seq_free = dict(SEQ0)
    hwdge_free = 0.0
    dma_free = 0.0
    ready = {}
    for (name, bpp, eng) in dma_items:
        if eng == "gp":
            # SWDGE prep on the Pool engine (parallel to HWDGE)
            t_p0 = seq_free["gp"]
            t_p1 = t_p0 + 994.0 + 0.34 * 128
            seq_free["gp"] = t_p1
            t_h1 = t_p1
        else:
            t_seq = seq_free[eng]
            t_h0 = max(t_seq, hwdge_free)
            t_h1 = t_h0 + HWD[eng]
            hwdge_free = t_h1
            seq_free[eng] = t_h1        # SEQ blocked until HWDGE done
        t_x0 = max(t_h1 + 650.0, dma_free)
        dur = bpp * 128 / 16.0 / 22.5 * (2.0 if bpp < 512 else 1.0)
        t_x1 = t_x0 + dur
        dma_free = t_x1
        ready[name] = t_x1 + 900.0

    def piece_segs(k0, k1):
        return [s for s in segs if k0 <= s["chunk"] < k1]

    last_seg_of_tile = {}
    for (k0, k1) in pieces:
        for s in piece_segs(k0, k1):
            last_seg_of_tile[s["tile"]] = id(s)
    tile_ready = {}
    for i, (k0, k1) in enumerate(pieces):
        for s in piece_segs(k0, k1):
            tile_ready[s["tile"]] = ready[f"p{i}"]

    with tile.TileContext(nc) as tc:
        with (
            nc.allow_low_precision(reason="fp8/fp16 pipeline validated vs 2e-2 tol"),
            tc.tile_pool(name="sb", bufs=1) as sb,
            tc.tile_pool(name="psQ", bufs=2, space="PSUM") as psQ,
            tc.tile_pool(name="psC", bufs=5, space="PSUM") as psC,
            tc.tile_pool(name="ps", bufs=1, space="PSUM") as ps,
        ):
            def slot(k):
                return pk_sb[:, k * 128:(k + 1) * 128]

            # ---- input DMAs (same order as the timeline model)
            pk_sb = sb.tile([P, PKW], F16, tag="pk_sb")
            a_sb = sb.tile([P, aw], F16, tag="a_sb")
            g8 = sb.tile([P, nchk, 128], F8, tag="g8")
            eng_map = dict(sp=nc.sync, act=nc.scalar, gp=nc.gpsimd)

            def piece_dma(i):
                k0, k1 = pieces[i]
                eng_map[piece_eng[i]].dma_start(
                    out=g8[:, k0:k1, :],
                    in_=tab8[:, k0 * 128:k1 * 128].rearrange(
                        "p (k f) -> p k f", f=128))

            nc.sync.dma_start(out=a_sb[:], in_=amat[:, :])
            piece_dma(0)
            nc.scalar.dma_start(out=pk_sb[:], in_=pk1[:, :])
            for i in range(1, len(pieces)):
                piece_dma(i)

            pair_T = pk_sb[:, CW:CW + 512]

            # prime the single ACT table set (exp+ln) at t=0
            warm = sb.tile([P, 1], F32, tag="warm")
            nc.vector.memset(warm[:], 1.0)
            warm2 = sb.tile([P, 2], F32, tag="warm2")
            nc.scalar.activation(warm2[:, 0:1], warm[:], AF.Ln)
            nc.scalar.activation(warm2[:, 1:2], warm[:], AF.Exp)

            paggT = [psQ.tile([P, 512], F32, tag="pagg", name=f"paggT{i}")
                     for i in range(2)]
            pagg = [paggT[i // 2][:, (i % 2) * 256:(i % 2) * 256 + 256]
                    for i in range(4)]
            nc.vector.memset(paggT[0][:], 0.0)
            nc.vector.memset(paggT[1][:], 0.0)

            def emit_piece_segs(i):
                k0, k1 = pieces[i]
                with tc.tile_wait_until(ready[f"p{i}"] / 1e6):
                    for s in piece_segs(k0, k1):
                        q = s["tile"]
                        nc.tensor.matmul(
                            out=paggT[q // 2][:, s["lo"] - (q // 2) * 512:
                                              s["hi"] - (q // 2) * 512],
                            lhsT=g8[:, s["chunk"], :],
                            rhs=a_sb[:, s["acol"]: s["acol"] + s["hi"] - s["lo"]],
                            start=False, stop=(last_seg_of_tile[q] == id(s)),
                            skip_group_check=True)

            # ---- early dense from pair_T
            kt = sb.tile([P, 512], F16, tag="kt")
            qt = sb.tile([P, 512], F16, tag="qt")
            vf = sb.tile([P, 256], F16, tag="vf")
            acts = sb.tile([P, 256], F16, tag="acts")
            sqs = sb.tile([P, 256], F16, tag="sqs")
            self_half = sb.tile([P, 256], F16, tag="self_half")
            shv = sb.tile([P, 256], F16, tag="shv")      # self_half - vf
            basep = sb.tile([P, 256], F16, tag="basep")  # 0.9*self_half + 0.1*vf
            qd = sb.tile([P, 256], F16, tag="qd")
            pd = sb.tile([P, 512], F16, tag="pd")
            eneg = sb.tile([P, 512], F16, tag="eneg")
            wden = sb.tile([P, 512], F16, tag="wden")
            wgt = sb.tile([P, 512], F16, tag="wgt")      # [waa 256 | wfa 256]
            cseq = sb.tile([P, 512], F16, tag="cseq")    # [agg | ff] nsum coef
            dseq = sb.tile([P, 512], F16, tag="dseq")    # [agg | ff] offset
            wsh = sb.tile([P, 512], F16, tag="wsh")
            t_pk = ready["pk1"]
            with tc.tile_wait_until(t_pk / 1e6):
                kt_ps = psC.tile([P, 512], F32, tag="psC", name="kt_ps")
                nc.tensor.matmul(out=kt_ps[:], lhsT=slot(S_WK), rhs=pair_T,
                                 start=True, stop=True)
                nc.scalar.copy(kt[:], kt_ps[:])
                qt_ps = psC.tile([P, 512], F32, tag="psC", name="qt_ps")
                nc.tensor.matmul(out=qt_ps[:], lhsT=slot(S_WQ), rhs=pair_T,
                                 start=True, stop=True)
                nc.scalar.copy(qt[:], qt_ps[:])
                vfs_ps = psC.tile([P, 512], F32, tag="psC", name="vfs_ps")
                nc.tensor.matmul(out=vfs_ps[:, 0:256], lhsT=slot(S_WVF),
                                 rhs=pair_T[:, 256:512], start=True, stop=True,
                                 skip_group_check=True)
                nc.tensor.matmul(out=vfs_ps[:, 256:512], lhsT=slot(S_WVA),
                                 rhs=pair_T[:, 0:256], start=True, stop=True,
                                 skip_group_check=True)
                nc.scalar.copy(vf[:], vfs_ps[:, 0:256])
                nc.vector.tensor_copy(acts[:], vfs_ps[:, 256:512])
                nc.vector.tensor_mul(sqs[:], acts[:], acts[:])
                nc.scalar.mul(self_half[:], acts[:], 0.5)
                nc.vector.tensor_sub(shv[:], self_half[:], vf[:])
                vf01 = sb.tile([P, 256], F16, tag="vf01")
                nc.scalar.mul(vf01[:], vf[:], 1.0 - RES_RATE)
                # basep = 0.9*self_half + 0.1*vf
                nc.vector.scalar_tensor_tensor(
                    out=basep[:], in0=self_half[:], scalar=RES_RATE,
                    in1=vf01[:], op0=OP.mult, op1=OP.add)
                # highway front (Pool helps: these are SBUF-only)
                nc.gpsimd.tensor_sub(qd[:], qt[:, 0:256], qt[:, 256:512])
                nc.gpsimd.tensor_mul(pd[:, 0:256], kt[:, 0:256], qd[:])
                nc.gpsimd.tensor_mul(pd[:, 256:512], kt[:, 256:512], qd[:])
                dif_ps = psC.tile([P, 512], F32, tag="psC", name="dif_ps")
                nc.tensor.matmul(out=dif_ps[:], lhsT=slot(S_ONESC), rhs=pd[:],
                                 start=True, stop=True)
                nc.scalar.activation(eneg[:], dif_ps[:], AF.Exp, scale=-1.0)
                nc.gpsimd.tensor_scalar_add(wden[:], eneg[:], 1.0)
                nc.vector.reciprocal(wgt[:], wden[:])
                # flattened final mix: pre = nsum*cseq + dseq (per-branch consts)
                nc.vector.tensor_scalar(out=cseq[:, 0:256], in0=wgt[:, 0:256],
                                        scalar1=0.05, scalar2=0.5 * RES_RATE,
                                        op0=OP.mult, op1=OP.add)
                nc.vector.tensor_scalar_mul(cseq[:, 256:512], wgt[:, 256:512], 0.05)
                nc.vector.tensor_tensor(
                    out=wsh[:].rearrange("p (k b) -> p k b", k=2),
                    in0=wgt[:].rearrange("p (k b) -> p k b", k=2),
                    in1=shv[:].rearrange("p b -> p b")[:, None, :].to_broadcast((P, 2, 256)),
                    op=OP.mult)
                nc.vector.scalar_tensor_tensor(
                    out=dseq[:, 0:256], in0=wsh[:, 0:256], scalar=1.0 - RES_RATE,
                    in1=basep[:], op0=OP.mult, op1=OP.add)
                nc.vector.scalar_tensor_tensor(
                    out=dseq[:, 256:512], in0=wsh[:, 256:512], scalar=1.0 - RES_RATE,
                    in1=vf[:], op0=OP.mult, op1=OP.add)

            # ---- persona chains: per-(node,mc) scalars run PACKED -----------
            # dest col c = 4b+m. Packed layout: partition = b (mod 128 within
            # a node-half i=pair), free col = m. den2/num are packed by PE
            # matmuls (m-strided lhsT, [*,1] rhs, self terms folded in as
            # accumulate matmuls); the tiny [128,4] scalar chain runs per
            # PAIR of quarters; coef is unpacked back to feature-replicated
            # form via PE transpose + four 1-partition broadcast matmuls.
            # nsum = Wva^T Σ_m coef_m·nraw_m (linearity: Wv after the mix),
            # with the Σ_m folded into 4 accumulated PE matmuls.
            nraw = sb.tile([P, 1024], F16, tag="nraw")
            sqn = sb.tile([P, 1024], F16, tag="sqn")
            tmul = sb.tile([P, 1024], F16, tag="tmul")   # m-major per quarter
            lden_pk = sb.tile([P, 2, 4], F16, tag="lden_pk")
            rden_pk = sb.tile([P, 2, 4], F16, tag="rden_pk")
            logit_pk = sb.tile([P, 2, 4], F16, tag="logit_pk")
            esm_pk = sb.tile([P, 2, 4], F16, tag="esm_pk")
            ssum_pk = sb.tile([P, 2], F16, tag="ssum_pk")
            rsum_pk = sb.tile([P, 2], F16, tag="rsum_pk")
            coef_pk = sb.tile([P, 2, 4], F16, tag="coef_pk")
            coefT = sb.tile([P, 2, 512], F16, tag="coefT")   # row 0 live
            ones_col = slot(S_ONES)[:, 0:1]
            wnum_col = slot(S_MUHI)[:, 0:1]
            wsmu_col = slot(S_MULO)[:, 0:1]
            ones_row = slot(S_ONES)[0:1, :]

            # pair PSUM tiles (alloc order matters for psC buf recycling)
            actn2 = [psC.tile([P, 512], F32, tag="psC", name=f"actn2_{h}")
                     for h in range(2)]
            misc = [psC.tile([P, 512], F32, tag="psC", name=f"misc_{h}")
                    for h in range(2)]
            coefR = [psC.tile([P, 512], F32, tag="psC", name=f"coefR_{h}")
                     for h in range(2)]
            ct_ps = [ps.tile([P, 512], F16, tag="ps_ct", name=f"ct_{h}", bufs=1)
                     for h in range(2)]

            nw_h = [sb.tile([P, 2, 128], F16, tag=f"nw{h}", name=f"nw{h}") for h in range(2)]
            pre_h = [sb.tile([P, 256], F16, tag=f"pre{h}", name=f"pre{h}") for h in range(2)]
            ep_h = [sb.tile([P, 256], F16, tag=f"ep{h}", name=f"ep{h}") for h in range(2)]
            rp_h = [sb.tile([P, 256], F16, tag=f"rp{h}", name=f"rp{h}") for h in range(2)]
            out_h = [sb.tile([P, 256], F16, tag=f"out{h}", name=f"out{h}") for h in range(2)]

            EMAP = {'a': nc.scalar, 'd': nc.vector, 'g': nc.gpsimd}

            def quarter_stages(q):
                qs = slice(q * 256, (q + 1) * 256)    # dest cols of quarter
                h = q // 2
                actn_ps = actn2[h][:, (q % 2) * 256:(q % 2) * 256 + 256]
                tq = tmul[:, qs].rearrange("p (m b) -> p m b", m=MC)
                nsq = misc[h][:, 256 + (q % 2) * 64: 256 + (q % 2) * 64 + 64]

                def st_nraw():
                    if ENG['nraw'][q] == 'a':
                        nc.scalar.copy(nraw[:, qs], pagg[q])
                    else:
                        nc.vector.tensor_copy(nraw[:, qs], pagg[q])

                def st_mm1():
                    nc.tensor.matmul(out=actn_ps, lhsT=slot(S_WVA),
                                     rhs=nraw[:, qs], start=True, stop=True,
                                     skip_group_check=True)

                def st_sqn():
                    if ENG['sqn'][q] == 'd':
                        nc.vector.tensor_mul(sqn[:, qs], actn_ps, actn_ps)
                    else:
                        nc.scalar.activation(sqn[:, qs], actn_ps, AF.Square)

                def st_tmul():
                    # tmul (m-major) = coef (replicated) * nraw
                    EMAP[ENG['tmul'][q]].tensor_tensor(
                        out=tq,
                        in0=coefR[h][:].rearrange("p (m b) -> p m b", b=128)
                            [:, :, (q % 2) * 64:(q % 2) * 64 + 64],
                        in1=nraw[:, qs].rearrange("p (b m) -> p m b", m=MC),
                        op=OP.mult)

                def st_nsmm():
                    # nsum = Wva^T Σ_m tmul_m  (4 accumulated matmuls)
                    for m in range(MC):
                        nc.tensor.matmul(out=nsq, lhsT=slot(S_WVA),
                                         rhs=tq[:, m, :], start=(m == 0),
                                         stop=(m == MC - 1),
                                         skip_group_check=True)

                return [(0, st_nraw), (350, st_mm1), (700, st_sqn),
                        (3350, st_tmul), (3800, st_nsmm)]

            def pair_stages(h):
                hs = slice(h * 512, (h + 1) * 512)
                pk_den = misc[h][:, 0:4]
                pk_num = misc[h][:, 4:8]
                nsum_ps = misc[h][:, 256:384]

                def st_pack():
                    sqn_m = sqn[:, hs].rearrange("p (b m) -> p m b", m=MC)
                    nraw_m = nraw[:, hs].rearrange("p (b m) -> p m b", m=MC)
                    half = slice(h * 128, (h + 1) * 128)
                    for m in range(MC):
                        nc.tensor.matmul(out=pk_den[:, m:m + 1], lhsT=sqn_m[:, m, :],
                                         rhs=ones_col, start=True, stop=False,
                                         skip_group_check=True)
                        nc.tensor.matmul(out=pk_den[:, m:m + 1], lhsT=sqs[:, half],
                                         rhs=ones_col, start=False, stop=True,
                                         skip_group_check=True)
                        nc.tensor.matmul(out=pk_num[:, m:m + 1], lhsT=nraw_m[:, m, :],
                                         rhs=wnum_col, start=True, stop=False,
                                         skip_group_check=True)
                        nc.tensor.matmul(out=pk_num[:, m:m + 1], lhsT=pair_T[:, half],
                                         rhs=wsmu_col, start=False, stop=True,
                                         skip_group_check=True)

                def st_lden():
                    nc.scalar.activation(lden_pk[:, h], pk_den, AF.Ln)

                def st_rden():
                    nc.scalar.activation(rden_pk[:, h], lden_pk[:, h], AF.Exp,
                                         scale=-0.5)

                def st_logit():
                    nc.vector.tensor_mul(logit_pk[:, h], pk_num, rden_pk[:, h])

                def st_esm():
                    nc.scalar.activation(esm_pk[:, h], logit_pk[:, h], AF.Exp)

                def st_coef():
                    nc.vector.reduce_sum(out=ssum_pk[:, h:h + 1],
                                         in_=esm_pk[:, h],
                                         axis=mybir.AxisListType.X)
                    nc.vector.reciprocal(rsum_pk[:, h:h + 1], ssum_pk[:, h:h + 1])
                    nc.vector.tensor_tensor(
                        out=coef_pk[:, h], in0=esm_pk[:, h],
                        in1=rsum_pk[:, h:h + 1].to_broadcast((P, MC)), op=OP.mult)

                def st_tr():
                    # one [128,1]->[1,128] transpose per m: all rows land on
                    # partition 0 (PE operands need base partition 0/32/64)
                    for m in range(MC):
                        nc.tensor.transpose(
                            ct_ps[h][0:1, m * 128:(m + 1) * 128],
                            coef_pk[:, h, m:m + 1], slot(S_ID))

                def st_ctcp():
                    nc.vector.tensor_copy(coefT[0:1, h, :], ct_ps[h][0:1, :])

                def st_unpack():
                    for m in range(MC):
                        nc.tensor.matmul(
                            out=coefR[h][:, m * 128:(m + 1) * 128],
                            lhsT=ones_row, rhs=coefT[0:1, h, m * 128:(m + 1) * 128],
                            start=True, stop=True, skip_group_check=True)

                def st_nw():
                    EMAP[ENG['nw'][h]].tensor_tensor(
                        out=nw_h[h][:],
                        in0=cseq[:].rearrange("p (k b) -> p k b", k=2)
                            [:, :, h * 128:(h + 1) * 128],
                        in1=nsum_ps[:, None, :].to_broadcast((P, 2, 128)),
                        op=OP.mult)

                def st_pre():
                    EMAP[ENG['pre'][h]].tensor_tensor(
                        out=pre_h[h][:].rearrange("p (k b) -> p k b", k=2),
                        in0=nw_h[h][:],
                        in1=dseq[:].rearrange("p (k b) -> p k b", k=2)
                            [:, :, h * 128:(h + 1) * 128],
                        op=OP.add)

                def st_elu():
                    # ELU: [relu(x) - 1] + min(exp(x), 1)
                    nc.scalar.activation(ep_h[h][:], pre_h[h][:], AF.Exp)
                    EMAP[ENG['rp'][h]].tensor_scalar(
                        out=rp_h[h][:], in0=pre_h[h][:], scalar1=0.0,
                        scalar2=-1.0, op0=OP.max, op1=OP.add)

                def st_out():
                    EMAP[ENG['stt'][h]].scalar_tensor_tensor(
                        out=out_h[h][:], in0=ep_h[h][:], scalar=1.0,
                        in1=rp_h[h][:], op0=OP.min, op1=OP.add)
                    nc.sync.dma_start(
                        out=out_t[:, :, h * 128:(h + 1) * 128].rearrange(
                            "c d b -> d c b"),
                        in_=out_h[h][:].rearrange("p (c b) -> p c b", b=128))

                return [(950, st_pack), (1350, st_lden), (1650, st_rden),
                        (1950, st_logit), (2250, st_esm), (2550, st_coef),
                        (2750, st_tr), (3000, st_ctcp), (3250, st_unpack),
                        (4150, st_nw), (4450, st_pre), (4750, st_elu),
                        (5150, st_out)]

            for i in range(len(pieces)):
                emit_piece_segs(i)
            f = ENG.get('soff', 1.0)
            chain_ops = []
            for q in range(4):
                for (off, fn) in quarter_stages(q):
                    anchor = tile_ready[q] if off < 3000 else tile_ready[q // 2 * 2 + 1]
                    chain_ops.append((anchor + off * f, fn))
            for h in range(2):
                for (off, fn) in pair_stages(h):
                    chain_ops.append((tile_ready[2 * h + 1] + off * f, fn))
            chain_ops.sort(key=lambda x: x[0])
            for (t_est, fn) in chain_ops:
                with tc.tile_wait_until(t_est / 1e6):
                    fn()

    # force the single combined exp+ln table set during the CFG pass,
    # then restore the true act_info.json index on the emitted loads
    orig_fn = bacc.get_activation_tables
    bacc.get_activation_tables = lambda arch: {target: orig_tables[target]}
    try:
        nc.compile()
    finally:
        bacc.get_activation_tables = orig_fn
    for blk in nc.m.functions[0].blocks:
        for ins in blk.instructions:
            if isinstance(ins, mybir.InstLoadActFuncSet):
                ins.act_func_set_id = target_id
    return nc


# --------------------------------------------------------------------------
# numpy simulation of the device pipeline (validates preprocessing + math)
# --------------------------------------------------------------------------

def numpy_simulate(inputs, plan, percore):
    cmat = make_consts(inputs).astype(np.float32)
    outs_a, outs_f = [], []
    for c in range(NCORES):
        pc = percore[c]
        g = pc["tab8"].astype(np.float32).reshape(P, plan["nchk"], 128)
        pair_T = pc["pairT"].astype(np.float32)
        pagg = np.zeros((4, P, 256), np.float32)
        for s in plan["segs"]:
            G = g[:, s["chunk"], :]
            A = pc["amat"].astype(np.float32)[:, s["acol"]: s["acol"] + s["hi"] - s["lo"]]
            pagg[s["tile"]][:, s["lo"] - s["tile"] * 256: s["hi"] - s["tile"] * 256] += G.T @ A
        neigh_rawT = np.concatenate(list(pagg), axis=1)
        Wva = cmat[:, S_WVA * 128:(S_WVA + 1) * 128]
        Wvf = cmat[:, S_WVF * 128:(S_WVF + 1) * 128]
        Wk = cmat[:, S_WK * 128:(S_WK + 1) * 128]
        Wq = cmat[:, S_WQ * 128:(S_WQ + 1) * 128]
        actn = Wva.T @ neigh_rawT                     # [128, 1024]
        acts = Wva.T @ pair_T[:, 0:256]               # [128, 256] self
        vf = Wvf.T @ pair_T[:, 256:512]
        kt = Wk.T @ pair_T
        qt = Wq.T @ pair_T
        n2 = (actn * actn).sum(0)
        s2 = (acts * acts).sum(0)
        w_num = cmat[:, S_MUHI * 128:S_MUHI * 128 + 1]
        w_smu = cmat[:, S_MULO * 128:S_MULO * 128 + 1]
        nmu = (w_num * neigh_rawT).sum(0)
        smu = (w_smu * pair_T[:, 0:256]).sum(0)
        den2 = n2 + np.repeat(s2, MC)
        numv = nmu + np.repeat(smu, MC)
        logit = numv / np.sqrt(den2)
        e = np.exp(logit).reshape(BC, MC)
        coef = e / e.sum(1, keepdims=True)
        neighT = actn.reshape(P, BC, MC)
        nsum = (neighT * coef[None]).sum(-1)
        vmid = 0.5 * (acts + nsum)
        saa = (kt[:, 0:256] * qt[:, 0:256]).sum(0) / DOUT
        saf = (kt[:, 0:256] * qt[:, 256:512]).sum(0) / DOUT
        sfa = (kt[:, 256:512] * qt[:, 0:256]).sum(0) / DOUT
        sff = (kt[:, 256:512] * qt[:, 256:512]).sum(0) / DOUT
        waa = 1.0 / (1.0 + np.exp(-(saa - saf)))
        wfa = 1.0 / (1.0 + np.exp(-(sfa - sff)))
        dd = vmid - vf
        new0 = vf + waa[None] * dd
        new1 = vf + wfa[None] * dd
        pre0 = RES_RATE * vmid + (1 - RES_RATE) * new0
        pre1 = RES_RATE * vf + (1 - RES_RATE) * new1
        elu = lambda x: np.where(x > 0, x, np.exp(np.minimum(x, 0)) - 1)
        outs_a.append(elu(pre0).T)
        outs_f.append(elu(pre1).T)
    return np.concatenate(outs_a, 0), np.concatenate(outs_f, 0)


# --------------------------------------------------------------------------
# public entry point
# --------------------------------------------------------------------------

_module_cache = {}
_last_results = None


def _plan_signature(plan):
    return (plan["nchk"], plan["aw"], plan["pieces"],
            tuple((s["chunk"], s["tile"], s["lo"], s["hi"], s["acol"])
                  for s in plan["segs"]))


def kernel(**inputs):
    plan, percore = preprocess(inputs)
    sig = _plan_signature(plan)
    if sig not in _module_cache:
        _module_cache[sig] = build_module(plan)
    nc = _module_cache[sig]

    cmat = make_consts(inputs)
    in_maps = []
    for c in range(NCORES):
        pc = percore[c]
        pk1 = np.concatenate([cmat, pc["pairT"]], axis=1)
        in_maps.append({
            "pk1": np.ascontiguousarray(pk1),
            "amat": pc["amat"],
            "tab8": pc["tab8"],
        })
    res = run_bass_kernel_spmd(nc, in_maps, core_ids=list(range(NCORES)))
    global _last_results
    _last_results = res
    agg_out = np.concatenate(
        [res.results[c]["out"][0].astype(np.float32).T for c in range(NCORES)], axis=0)
    ff_out = np.concatenate(
        [res.results[c]["out"][1].astype(np.float32).T for c in range(NCORES)], axis=0)
    return agg_out, ff_out


# revision 22
# speedup vs baseline: 1.0815x; 1.0815x over previous
"""Trainium2 Bass kernel for nn_CFAggregator (GNN message passing).

Strategy (B-sharded data parallel over 8 cores, no collectives):
  - Host: all indexed loads are pre-staged per core. The edge feature rows
    (agg_table[unique_ids[col_idx]]) are laid out in PE-ready chunk-slot
    order as an fp8 [128, nchk*128] tensor (partition = slot-in-chunk), so
    the on-device "gather" is one contiguous full-bandwidth DMA. The self
    features are staged pre-transposed (pair_T [feat, node]), removing the
    on-device PE transposes. All synthesized constants (ones, 1/DOUT, the
    mu_w vectors pre-folded through Wv_agg) are materialized into a single
    packed fp16 tensor. The dedup'd edge weights (mask .set() + 1/cnt) ride
    in a block one-hot A matrix (fp16) mapping slots to dest columns.
  - Device: plain DMAs (HWDGE spread across SP/ACT/DVE queues, tab8 split
    into pieces to pipeline with PE), PE accumulates G_chunk^T @ A_chunk
    into four 256-column PSUM quarters. Each quarter runs an independent
    chain (Wv matmul, stats matmuls with 0-stride broadcast self fold-in,
    1/sqrt via exp(-0.5 ln) on a single forced exp+ln activation table,
    softmax over MC, highway attention mix, ELU) with ops globally ordered
    by estimated start time and balanced across ACT/DVE/Pool; per-quarter
    output DMAs overlap the remaining chains.
The host only performs index math, dtype conversion, and row restaging;
all arithmetic on feature values happens on-device.
"""

import numpy as np
import ml_dtypes

import concourse.bass as bass
import concourse.bacc as bacc
import concourse.tile as tile
from concourse import mybir
from concourse.bass_utils import run_bass_kernel_spmd

F32 = mybir.dt.float32
F16 = mybir.dt.float16
F8 = mybir.dt.float8e4
I32 = mybir.dt.int32
AF = mybir.ActivationFunctionType
OP = mybir.AluOpType
NPF16 = np.float16
NPF8 = ml_dtypes.float8_e4m3fn

# problem dims (hardcoded per contract)
B, MC, U, N, DIN, DOUT, E = 2048, 4, 20000, 100000, 128, 128, 65536
RES_RATE = 0.9
NCORES = 8
BC = B // NCORES          # 256 nodes per core
DEST = BC * MC            # 1024 destination columns per core
P = 128

# consts tile slots (each [128, 128] fp16)
(S_WK, S_WQ, S_WVA, S_WVF, S_ONES, S_ONESC, S_MUHI, S_MULO, S_ID) = range(9)
NSLOT = 9
CW = NSLOT * 128          # consts width
PKW = CW + 512            # packed: consts | pair_T
# per-quarter engine strings (index q): 'a'=ACT, 'd'=DVE, 'g'=Pool
ENG = dict(nraw='dada', sqn='aaaa', tmul='dddd', nw='dd', pre='gg',
           rp='gg', stt='dd', soff=1.0)


# --------------------------------------------------------------------------
# host-side preprocessing (index math + row restaging only)
# --------------------------------------------------------------------------

def preprocess(inputs):
    """Build per-core staged tensors + seg plan. Returns (plan, percore)."""
    nodes = np.asarray(inputs["nodes"]).astype(np.int64)
    unique_ids = np.asarray(inputs["unique_ids"]).astype(np.int64)
    row_idx = np.asarray(inputs["row_idx"]).astype(np.int64)
    layer_idx = np.asarray(inputs["layer_idx"]).astype(np.int64)
    col_idx = np.asarray(inputs["col_idx"]).astype(np.int64)

    eff = unique_ids[col_idx]                       # table row per edge
    # dedup (b, layer, col) triples: .set() counts duplicates once
    key = (row_idx * MC + layer_idx) * U + col_idx
    uniq_keys, first_pos = np.unique(key, return_index=True)
    keep = np.zeros(E, bool)
    keep[first_pos] = True
    grp_of_uniq = uniq_keys // U
    cnt = np.bincount(grp_of_uniq, minlength=B * MC)
    grp = row_idx * MC + layer_idx
    w = np.where(keep, 1.0 / np.maximum(cnt[grp], 1), 0.0).astype(np.float32)
    dest_all = (row_idx % BC) * MC + layer_idx      # core-local dest column

    # per-core dest-sorted edge stream
    core_lists = []
    for c in range(NCORES):
        sel = (row_idx >= c * BC) & (row_idx < (c + 1) * BC)
        order = np.argsort(dest_all[sel], kind="stable")
        core_lists.append((eff[sel][order], dest_all[sel][order], w[sel][order]))

    mx = max(len(cl[0]) for cl in core_lists)
    nchk = (mx + 127) // 128
    cap = nchk * 128

    core_streams = []   # (idx, dest, w) padded to cap, quantile-aligned
    for c in range(NCORES):
        idxs, dests, ws = core_lists[c]
        n = len(idxs)
        s_idx = np.full(cap, -1, np.int64)
        s_dst = np.full(cap, -1, np.int64)
        s_w = np.zeros(cap, np.float32)
        bnd = np.round(np.arange(nchk + 1) * n / nchk).astype(np.int64)
        for k in range(nchk):
            e0, e1 = bnd[k], bnd[k + 1]
            s_idx[k * 128:k * 128 + e1 - e0] = idxs[e0:e1]
            s_dst[k * 128:k * 128 + e1 - e0] = dests[e0:e1]
            s_w[k * 128:k * 128 + e1 - e0] = ws[e0:e1]
        core_streams.append((s_idx, s_dst, s_w))

    # per-chunk dest spans = union over cores of real dests
    spans = []
    for k in range(nchk):
        lo, hi = DEST, 0
        for c in range(NCORES):
            d = core_streams[c][1][k * 128:(k + 1) * 128]
            d = d[d >= 0]
            if len(d):
                lo = min(lo, int(d.min()))
                hi = max(hi, int(d.max()) + 1)
        if hi <= lo:
            lo, hi = -1, -1
        spans.append((k, lo, hi))

    # segments: split spans at 256-column quarter-tile boundaries
    segs = []
    acol = 0
    for (k, lo, hi) in spans:
        if lo < 0:
            continue
        for t in range(4):
            b0, b1 = t * 256, (t + 1) * 256
            s0, s1 = max(lo, b0), min(hi, b1)
            if s1 > s0:
                segs.append(dict(chunk=k, tile=t, lo=s0, hi=s1,
                                 acol=acol + (s0 - lo)))
        acol += hi - lo
    aw = max(acol, 1)

    # tab8 DMA pieces: split at quarter-tile boundaries so tile q waits
    # only on piece q (dest-sorted chunks make this near-exact)
    qchunk = []
    for t in range(4):
        ks = [s["chunk"] for s in segs if s["tile"] == t]
        qchunk.append(max(ks) + 1 if ks else None)
    pb = [0]
    for t in range(4):
        nxt = qchunk[t] if qchunk[t] is not None else pb[-1]
        nxt = max(nxt, pb[-1])
        pb.append(min(nxt, nchk))
    pb[-1] = nchk
    pieces = tuple((int(pb[i]), int(pb[i + 1])) for i in range(4)
                   if pb[i + 1] > pb[i])

    plan = dict(nchk=nchk, aw=aw, segs=segs, pieces=pieces)

    agg8 = np.asarray(inputs["agg_table"], np.float32).astype(NPF8)
    agg16 = np.asarray(inputs["agg_table"], np.float32).astype(NPF16)
    ff16 = np.asarray(inputs["ff_table"], np.float32).astype(NPF16)

    span_acol = {}
    ac = 0
    for (k, lo, hi) in spans:
        span_acol[k] = (ac, lo)
        if lo >= 0:
            ac += hi - lo

    percore = []
    for c in range(NCORES):
        s_idx, s_dst, s_w = core_streams[c]
        # staged edge rows, PE layout: [slot-in-chunk (partition), chunk*128+feat]
        tab8 = np.zeros((P, nchk * 128), NPF8)
        rows = agg8[np.maximum(s_idx, 0)]           # [cap, 128]
        rows[s_idx < 0] = 0
        tab8[:] = rows.reshape(nchk, 128, 128).transpose(1, 0, 2).reshape(P, -1)
        # A matrix
        amat = np.zeros((P, aw), NPF16)
        for k in range(nchk):
            a0, lo = span_acol[k]
            if lo < 0:
                continue
            sl = slice(k * 128, (k + 1) * 128)
            real = s_dst[sl] >= 0
            pp = np.nonzero(real)[0]
            amat[pp, a0 + s_dst[sl][pp] - lo] = s_w[sl][pp].astype(NPF16)
        # pair_T staged pre-transposed: [feat, h*128+p] (agg cols 0:256, ff 256:512)
        nd = nodes[c * BC:(c + 1) * BC]
        pairT = np.zeros((P, 512), NPF16)
        pairT[:, 0:256] = agg16[nd].T
        pairT[:, 256:512] = ff16[nd].T
        percore.append(dict(tab8=tab8, amat=amat, pairT=pairT))

    return plan, percore


def make_consts(inputs):
    """[128, CW] fp16 consts block (weights + synthesized constants)."""
    c = np.zeros((P, CW), NPF16)
    c[:, S_WK * 128:(S_WK + 1) * 128] = np.asarray(inputs["Wk"], np.float32)
    c[:, S_WQ * 128:(S_WQ + 1) * 128] = np.asarray(inputs["Wq"], np.float32)
    c[:, S_WVA * 128:(S_WVA + 1) * 128] = np.asarray(inputs["Wv_agg"], np.float32)
    c[:, S_WVF * 128:(S_WVF + 1) * 128] = np.asarray(inputs["Wv_ff"], np.float32)
    c[:, S_ONES * 128:(S_ONES + 1) * 128] = 1.0
    c[:, S_ONESC * 128:(S_ONESC + 1) * 128] = 1.0 / DOUT
    mu = np.asarray(inputs["mu_w"]).astype(np.float32).reshape(2, DOUT).T
    wva = np.asarray(inputs["Wv_agg"], np.float32)
    w_num = (wva @ mu[:, 1]).astype(NPF16)          # neighbor vector
    w_smu = (wva @ mu[:, 0]).astype(NPF16)          # self vector
    c[:, S_MUHI * 128:(S_MUHI + 1) * 128] = w_num[:, None]
    c[:, S_MULO * 128:(S_MULO + 1) * 128] = w_smu[:, None]
    c[:, S_ID * 128:(S_ID + 1) * 128] = np.eye(P, dtype=NPF16)
    return c


# --------------------------------------------------------------------------
# device module
# --------------------------------------------------------------------------

def build_module(plan):
    nchk = plan["nchk"]
    aw = plan["aw"]
    segs = plan["segs"]
    pieces = plan["pieces"]

    nc = bacc.Bacc("TRN2", target_bir_lowering=False, debug=False,
                   num_devices=NCORES)

    import concourse.hw_specs as hw_specs
    orig_tables = hw_specs.get_activation_tables(nc.m.arch)
    tnames = list(orig_tables)
    target = next(n for n in tnames
                  if {AF.Exp, AF.Ln, AF.Copy} <= orig_tables[n])
    target_id = tnames.index(target)

    pk1 = nc.dram_tensor("pk1", [P, PKW], F16, kind="ExternalInput")
    amat = nc.dram_tensor("amat", [P, aw], F16, kind="ExternalInput")
    tab8 = nc.dram_tensor("tab8", [P, nchk * 128], F8, kind="ExternalInput")
    out_t = nc.dram_tensor("out", [P, 2, 2, 128], F16, kind="ExternalOutput")

    # ---- DMA timeline model (for scheduler wait hints) ------------------
    # transfer order: pk1 (early-dense dep) first, then amat, then pieces
    piece_eng = ["gp", "sp", "act", "gp", "sp", "act"]
    dma_items = [("pk1", PKW * 2, "sp"), ("amat", aw * 2, "act")]
    for i, (k0, k1) in enumerate(pieces):
        dma_items.append((f"p{i}", (k1 - k0) * 128, piece_eng[i]))
    HWD = dict(sp=625.0, act=632.0)
    SEQ0 = dict(sp=25.0, act=32.0, gp=61.0)
    seq_free = dict(SEQ0)
    hwdge_free = 0.0
    dma_free = 0.0
    ready = {}
    for (name, bpp, eng) in dma_items:
        if eng == "gp":
            # SWDGE prep on the Pool engine (parallel to HWDGE)
            t_p0 = seq_free["gp"]
            t_p1 = t_p0 + 994.0 + 0.34 * 128
            seq_free["gp"] = t_p1
            t_h1 = t_p1
        else:
            t_seq = seq_free[eng]
            t_h0 = max(t_seq, hwdge_free)
            t_h1 = t_h0 + HWD[eng]
            hwdge_free = t_h1
            seq_free[eng] = t_h1        # SEQ blocked until HWDGE done
        t_x0 = max(t_h1 + 650.0, dma_free)
        dur = bpp * 128 / 16.0 / 22.5 * (2.0 if bpp < 512 else 1.0)
        t_x1 = t_x0 + dur
        dma_free = t_x1
        ready[name] = t_x1 + 900.0

    def piece_segs(k0, k1):
        return [s for s in segs if k0 <= s["chunk"] < k1]

    last_seg_of_tile = {}
    for (k0, k1) in pieces:
        for s in piece_segs(k0, k1):
            last_seg_of_tile[s["tile"]] = id(s)
    tile_ready = {}
    for i, (k0, k1) in enumerate(pieces):
        for s in piece_segs(k0, k1):
            tile_ready[s["tile"]] = ready[f"p{i}"]

    with tile.TileContext(nc) as tc:
        with (
            nc.allow_low_precision(reason="fp8/fp16 pipeline validated vs 2e-2 tol"),
            tc.tile_pool(name="sb", bufs=1) as sb,
            tc.tile_pool(name="psQ", bufs=2, space="PSUM") as psQ,
            tc.tile_pool(name="psC", bufs=5, space="PSUM") as psC,
            tc.tile_pool(name="ps", bufs=1, space="PSUM") as ps,
        ):
            def slot(k):
                return pk_sb[:, k * 128:(k + 1) * 128]

            # ---- input DMAs (same order as the timeline model)
            pk_sb = sb.tile([P, PKW], F16, tag="pk_sb")
            a_sb = sb.tile([P, aw], F16, tag="a_sb")
            g8 = sb.tile([P, nchk, 128], F8, tag="g8")
            eng_map = dict(sp=nc.sync, act=nc.scalar, gp=nc.gpsimd)

            def piece_dma(i):
                k0, k1 = pieces[i]
                eng_map[piece_eng[i]].dma_start(
                    out=g8[:, k0:k1, :],
                    in_=tab8[:, k0 * 128:k1 * 128].rearrange(
                        "p (k f) -> p k f", f=128))

            nc.sync.dma_start(out=pk_sb[:], in_=pk1[:, :])
            nc.scalar.dma_start(out=a_sb[:], in_=amat[:, :])
            for i in range(len(pieces)):
                piece_dma(i)

            pair_T = pk_sb[:, CW:CW + 512]

            # prime the single ACT table set (exp+ln) at t=0
            warm = sb.tile([P, 1], F32, tag="warm")
            nc.vector.memset(warm[:], 1.0)
            warm2 = sb.tile([P, 2], F32, tag="warm2")
            nc.scalar.activation(warm2[:, 0:1], warm[:], AF.Ln)
            nc.scalar.activation(warm2[:, 1:2], warm[:], AF.Exp)

            paggT = [psQ.tile([P, 512], F32, tag="pagg", name=f"paggT{i}")
                     for i in range(2)]
            pagg = [paggT[i // 2][:, (i % 2) * 256:(i % 2) * 256 + 256]
                    for i in range(4)]
            nc.vector.memset(paggT[0][:], 0.0)
            nc.vector.memset(paggT[1][:], 0.0)

            def emit_piece_segs(i):
                k0, k1 = pieces[i]
                with tc.tile_wait_until(ready[f"p{i}"] / 1e6):
                    for s in piece_segs(k0, k1):
                        q = s["tile"]
                        nc.tensor.matmul(
                            out=paggT[q // 2][:, s["lo"] - (q // 2) * 512:
                                              s["hi"] - (q // 2) * 512],
                            lhsT=g8[:, s["chunk"], :],
                            rhs=a_sb[:, s["acol"]: s["acol"] + s["hi"] - s["lo"]],
                            start=False, stop=(last_seg_of_tile[q] == id(s)),
                            skip_group_check=True)

            # ---- early dense from pair_T
            kt = sb.tile([P, 512], F16, tag="kt")
            pdiff = sb.tile([P, 256], F16, tag="pdiff")
            vf = sb.tile([P, 256], F16, tag="vf")
            acts = sb.tile([P, 256], F16, tag="acts")
            sqs = sb.tile([P, 256], F16, tag="sqs")
            self_half = sb.tile([P, 256], F16, tag="self_half")
            shv = sb.tile([P, 256], F16, tag="shv")      # self_half - vf
            basep = sb.tile([P, 256], F16, tag="basep")  # 0.9*self_half + 0.1*vf
            pd = sb.tile([P, 512], F16, tag="pd")
            eneg = sb.tile([P, 512], F16, tag="eneg")
            wden = sb.tile([P, 512], F16, tag="wden")
            wgt = sb.tile([P, 512], F16, tag="wgt")      # [waa 256 | wfa 256]
            cseq = sb.tile([P, 512], F16, tag="cseq")    # [agg | ff] nsum coef
            dseq = sb.tile([P, 512], F16, tag="dseq")    # [agg | ff] offset
            wsh = sb.tile([P, 512], F16, tag="wsh")
            t_pk = ready["pk1"]
            with tc.tile_wait_until(t_pk / 1e6):
                # qd = Wq^T (pair_agg - pair_ff): diff before the matmul so
                # the qt copy drops off the critical chain entirely
                nc.vector.tensor_sub(pdiff[:], pair_T[:, 0:256],
                                     pair_T[:, 256:512])
                kt_ps = psC.tile([P, 512], F32, tag="psC", name="kt_ps")
                nc.tensor.matmul(out=kt_ps[:], lhsT=slot(S_WK), rhs=pair_T,
                                 start=True, stop=True)
                nc.scalar.copy(kt[:], kt_ps[:])
                qt_ps = psC.tile([P, 512], F32, tag="psC", name="qt_ps")
                qd_ps = qt_ps[:, 0:256]
                nc.tensor.matmul(out=qd_ps, lhsT=slot(S_WQ), rhs=pdiff[:],
                                 start=True, stop=True, skip_group_check=True)
                vfs_ps = psC.tile([P, 512], F32, tag="psC", name="vfs_ps")
                nc.tensor.matmul(out=vfs_ps[:, 0:256], lhsT=slot(S_WVF),
                                 rhs=pair_T[:, 256:512], start=True, stop=True,
                                 skip_group_check=True)
                nc.tensor.matmul(out=vfs_ps[:, 256:512], lhsT=slot(S_WVA),
                                 rhs=pair_T[:, 0:256], start=True, stop=True,
                                 skip_group_check=True)
                nc.scalar.copy(vf[:], vfs_ps[:, 0:256])
                nc.vector.tensor_copy(acts[:], vfs_ps[:, 256:512])
                nc.vector.tensor_mul(sqs[:], acts[:], acts[:])
                nc.scalar.mul(self_half[:], acts[:], 0.5)
                nc.gpsimd.tensor_sub(shv[:], self_half[:], vf[:])
                vf01 = sb.tile([P, 256], F16, tag="vf01")
                nc.scalar.mul(vf01[:], vf[:], 1.0 - RES_RATE)
                # basep = 0.9*self_half + 0.1*vf
                nc.vector.scalar_tensor_tensor(
                    out=basep[:], in0=self_half[:], scalar=RES_RATE,
                    in1=vf01[:], op0=OP.mult, op1=OP.add)
                # highway front
                nc.vector.tensor_mul(pd[:, 0:256], kt[:, 0:256], qd_ps)
                nc.vector.tensor_mul(pd[:, 256:512], kt[:, 256:512], qd_ps)
                dif_ps = psC.tile([P, 512], F32, tag="psC", name="dif_ps")
                nc.tensor.matmul(out=dif_ps[:], lhsT=slot(S_ONESC), rhs=pd[:],
                                 start=True, stop=True)
                nc.scalar.activation(eneg[:], dif_ps[:], AF.Exp, scale=-1.0)
                nc.vector.tensor_scalar_add(wden[:], eneg[:], 1.0)
                nc.vector.reciprocal(wgt[:], wden[:])
                # flattened final mix: pre = nsum*cseq + dseq (per-branch consts)
                nc.vector.tensor_scalar(out=cseq[:, 0:256], in0=wgt[:, 0:256],
                                        scalar1=0.05, scalar2=0.5 * RES_RATE,
                                        op0=OP.mult, op1=OP.add)
                nc.vector.tensor_scalar_mul(cseq[:, 256:512], wgt[:, 256:512], 0.05)
                nc.vector.tensor_tensor(
                    out=wsh[:].rearrange("p (k b) -> p k b", k=2),
                    in0=wgt[:].rearrange("p (k b) -> p k b", k=2),
                    in1=shv[:].rearrange("p b -> p b")[:, None, :].to_broadcast((P, 2, 256)),
                    op=OP.mult)
                nc.vector.scalar_tensor_tensor(
                    out=dseq[:, 0:256], in0=wsh[:, 0:256], scalar=1.0 - RES_RATE,
                    in1=basep[:], op0=OP.mult, op1=OP.add)
                nc.vector.scalar_tensor_tensor(
                    out=dseq[:, 256:512], in0=wsh[:, 256:512], scalar=1.0 - RES_RATE,
                    in1=vf[:], op0=OP.mult, op1=OP.add)

            # ---- persona chains: per-(node,mc) scalars run PACKED -----------
            # dest col c = 4b+m. Packed layout: partition = b (mod 128 within
            # a node-half i=pair), free col = m. den2/num are packed by PE
            # matmuls (m-strided lhsT, [*,1] rhs, self terms folded in as
            # accumulate matmuls); the tiny [128,4] scalar chain runs per
            # PAIR of quarters; coef is unpacked back to feature-replicated
            # form via PE transpose + four 1-partition broadcast matmuls.
            # nsum = Wva^T Σ_m coef_m·nraw_m (linearity: Wv after the mix),
            # with the Σ_m folded into 4 accumulated PE matmuls.
            nraw = sb.tile([P, 1024], F16, tag="nraw")
            sqn = sb.tile([P, 1024], F16, tag="sqn")
            tmul = sb.tile([P, 1024], F16, tag="tmul")   # m-major per quarter
            lden_pk = sb.tile([P, 2, 4], F16, tag="lden_pk")
            rden_pk = sb.tile([P, 2, 4], F16, tag="rden_pk")
            logit_pk = sb.tile([P, 2, 4], F16, tag="logit_pk")
            esm_pk = sb.tile([P, 2, 4], F16, tag="esm_pk")
            ssum_pk = sb.tile([P, 2], F16, tag="ssum_pk")
            rsum_pk = sb.tile([P, 2], F16, tag="rsum_pk")
            coef_pk = sb.tile([P, 2, 4], F16, tag="coef_pk")
            coefT = sb.tile([P, 2, 512], F16, tag="coefT")   # row 0 live
            ones_col = slot(S_ONES)[:, 0:1]
            wnum_col = slot(S_MUHI)[:, 0:1]
            wsmu_col = slot(S_MULO)[:, 0:1]
            ones_row = slot(S_ONES)[0:1, :]

            # pair PSUM tiles (alloc order matters for psC buf recycling)
            actn2 = [psC.tile([P, 512], F32, tag="psC", name=f"actn2_{h}")
                     for h in range(2)]
            misc = [psC.tile([P, 512], F32, tag="psC", name=f"misc_{h}")
                    for h in range(2)]
            coefR = [psC.tile([P, 512], F32, tag="psC", name=f"coefR_{h}")
                     for h in range(2)]
            ct_ps = [ps.tile([P, 512], F16, tag="ps_ct", name=f"ct_{h}", bufs=1)
                     for h in range(2)]

            nw_h = [sb.tile([P, 2, 128], F16, tag=f"nw{h}", name=f"nw{h}") for h in range(2)]
            pre_h = [sb.tile([P, 256], F16, tag=f"pre{h}", name=f"pre{h}") for h in range(2)]
            ep_h = [sb.tile([P, 256], F16, tag=f"ep{h}", name=f"ep{h}") for h in range(2)]
            rp_h = [sb.tile([P, 256], F16, tag=f"rp{h}", name=f"rp{h}") for h in range(2)]
            out_h = [sb.tile([P, 256], F16, tag=f"out{h}", name=f"out{h}") for h in range(2)]

            EMAP = {'a': nc.scalar, 'd': nc.vector, 'g': nc.gpsimd}

            def quarter_stages(q):
                qs = slice(q * 256, (q + 1) * 256)    # dest cols of quarter
                h = q // 2
                actn_ps = actn2[h][:, (q % 2) * 256:(q % 2) * 256 + 256]
                tq = tmul[:, qs].rearrange("p (m b) -> p m b", m=MC)
                nsq = misc[h][:, 256 + (q % 2) * 64: 256 + (q % 2) * 64 + 64]

                def st_nraw():
                    if ENG['nraw'][q] == 'a':
                        nc.scalar.copy(nraw[:, qs], pagg[q])
                    else:
                        nc.vector.tensor_copy(nraw[:, qs], pagg[q])

                def st_mm1():
                    nc.tensor.matmul(out=actn_ps, lhsT=slot(S_WVA),
                                     rhs=nraw[:, qs], start=True, stop=True,
                                     skip_group_check=True)

                def st_sqn():
                    if ENG['sqn'][q] == 'd':
                        nc.vector.tensor_mul(sqn[:, qs], actn_ps, actn_ps)
                    else:
                        nc.scalar.activation(sqn[:, qs], actn_ps, AF.Square)

                def st_tmul():
                    # tmul (m-major) = coef (replicated) * nraw
                    EMAP[ENG['tmul'][q]].tensor_tensor(
                        out=tq,
                        in0=coefR[h][:].rearrange("p (m b) -> p m b", b=128)
                            [:, :, (q % 2) * 64:(q % 2) * 64 + 64],
                        in1=nraw[:, qs].rearrange("p (b m) -> p m b", m=MC),
                        op=OP.mult)

                def st_nsmm():
                    # nsum = Wva^T Σ_m tmul_m  (4 accumulated matmuls)
                    for m in range(MC):
                        nc.tensor.matmul(out=nsq, lhsT=slot(S_WVA),
                                         rhs=tq[:, m, :], start=(m == 0),
                                         stop=(m == MC - 1),
                                         skip_group_check=True)

                return [(0, st_nraw), (350, st_mm1), (700, st_sqn),
                        (3350, st_tmul), (3800, st_nsmm)]

            def pair_stages(h):
                hs = slice(h * 512, (h + 1) * 512)
                pk_den = misc[h][:, 0:4]
                pk_num = misc[h][:, 4:8]
                nsum_ps = misc[h][:, 256:384]

                def st_pack():
                    sqn_m = sqn[:, hs].rearrange("p (b m) -> p m b", m=MC)
                    nraw_m = nraw[:, hs].rearrange("p (b m) -> p m b", m=MC)
                    half = slice(h * 128, (h + 1) * 128)
                    for m in range(MC):
                        nc.tensor.matmul(out=pk_den[:, m:m + 1], lhsT=sqn_m[:, m, :],
                                         rhs=ones_col, start=True, stop=False,
                                         skip_group_check=True)
                        nc.tensor.matmul(out=pk_den[:, m:m + 1], lhsT=sqs[:, half],
                                         rhs=ones_col, start=False, stop=True,
                                         skip_group_check=True)
                        nc.tensor.matmul(out=pk_num[:, m:m + 1], lhsT=nraw_m[:, m, :],
                                         rhs=wnum_col, start=True, stop=False,
                                         skip_group_check=True)
                        nc.tensor.matmul(out=pk_num[:, m:m + 1], lhsT=pair_T[:, half],
                                         rhs=wsmu_col, start=False, stop=True,
                                         skip_group_check=True)

                def st_lden():
                    nc.scalar.activation(lden_pk[:, h], pk_den, AF.Ln)

                def st_rden():
                    nc.scalar.activation(rden_pk[:, h], lden_pk[:, h], AF.Exp,
                                         scale=-0.5)

                def st_logit():
                    nc.vector.tensor_mul(logit_pk[:, h], pk_num, rden_pk[:, h])

                def st_esm():
                    nc.scalar.activation(esm_pk[:, h], logit_pk[:, h], AF.Exp)

                def st_coef():
                    nc.vector.reduce_sum(out=ssum_pk[:, h:h + 1],
                                         in_=esm_pk[:, h],
                                         axis=mybir.AxisListType.X)
                    nc.vector.reciprocal(rsum_pk[:, h:h + 1], ssum_pk[:, h:h + 1])
                    nc.vector.tensor_tensor(
                        out=coef_pk[:, h], in0=esm_pk[:, h],
                        in1=rsum_pk[:, h:h + 1].to_broadcast((P, MC)), op=OP.mult)

                def st_tr():
                    # one [128,1]->[1,128] transpose per m: all rows land on
                    # partition 0 (PE operands need base partition 0/32/64)
                    for m in range(MC):
                        nc.tensor.transpose(
                            ct_ps[h][0:1, m * 128:(m + 1) * 128],
                            coef_pk[:, h, m:m + 1], slot(S_ID))

                def st_ctcp():
                    nc.vector.tensor_copy(coefT[0:1, h, :], ct_ps[h][0:1, :])

                def st_unpack():
                    for m in range(MC):
                        nc.tensor.matmul(
                            out=coefR[h][:, m * 128:(m + 1) * 128],
                            lhsT=ones_row, rhs=coefT[0:1, h, m * 128:(m + 1) * 128],
                            start=True, stop=True, skip_group_check=True)

                def st_nw():
                    EMAP[ENG['nw'][h]].tensor_tensor(
                        out=nw_h[h][:],
                        in0=cseq[:].rearrange("p (k b) -> p k b", k=2)
                            [:, :, h * 128:(h + 1) * 128],
                        in1=nsum_ps[:, None, :].to_broadcast((P, 2, 128)),
                        op=OP.mult)

                def st_pre():
                    EMAP[ENG['pre'][h]].tensor_tensor(
                        out=pre_h[h][:].rearrange("p (k b) -> p k b", k=2),
                        in0=nw_h[h][:],
                        in1=dseq[:].rearrange("p (k b) -> p k b", k=2)
                            [:, :, h * 128:(h + 1) * 128],
                        op=OP.add)

                def st_elu():
                    # ELU: [relu(x) - 1] + min(exp(x), 1)
                    nc.scalar.activation(ep_h[h][:], pre_h[h][:], AF.Exp)
                    EMAP[ENG['rp'][h]].tensor_scalar(
                        out=rp_h[h][:], in0=pre_h[h][:], scalar1=0.0,
                        scalar2=-1.0, op0=OP.max, op1=OP.add)

                def st_out():
                    EMAP[ENG['stt'][h]].scalar_tensor_tensor(
                        out=out_h[h][:], in0=ep_h[h][:], scalar=1.0,
                        in1=rp_h[h][:], op0=OP.min, op1=OP.add)
                    nc.sync.dma_start(out=out_t[:, h], in_=out_h[h][:].rearrange(
                        "p (c b) -> p c b", b=128))

                return [(950, st_pack), (1350, st_lden), (1650, st_rden),
                        (1950, st_logit), (2250, st_esm), (2550, st_coef),
                        (2750, st_tr), (3000, st_ctcp), (3250, st_unpack),
                        (4150, st_nw), (4450, st_pre), (4750, st_elu),
                        (5150, st_out)]

            for i in range(len(pieces)):
                emit_piece_segs(i)
            f = ENG.get('soff', 1.0)
            chain_ops = []
            for q in range(4):
                for (off, fn) in quarter_stages(q):
                    anchor = tile_ready[q] if off < 3000 else tile_ready[q // 2 * 2 + 1]
                    chain_ops.append((anchor + off * f, fn))
            for h in range(2):
                for (off, fn) in pair_stages(h):
                    chain_ops.append((tile_ready[2 * h + 1] + off * f, fn))
            chain_ops.sort(key=lambda x: x[0])
            for (t_est, fn) in chain_ops:
                with tc.tile_wait_until(t_est / 1e6):
                    fn()

    # force the single combined exp+ln table set during the CFG pass,
    # then restore the true act_info.json index on the emitted loads
    orig_fn = bacc.get_activation_tables
    bacc.get_activation_tables = lambda arch: {target: orig_tables[target]}
    try:
        nc.compile()
    finally:
        bacc.get_activation_tables = orig_fn
    for blk in nc.m.functions[0].blocks:
        for ins in blk.instructions:
            if isinstance(ins, mybir.InstLoadActFuncSet):
                ins.act_func_set_id = target_id
    return nc


# --------------------------------------------------------------------------
# numpy simulation of the device pipeline (validates preprocessing + math)
# --------------------------------------------------------------------------

def numpy_simulate(inputs, plan, percore):
    cmat = make_consts(inputs).astype(np.float32)
    outs_a, outs_f = [], []
    for c in range(NCORES):
        pc = percore[c]
        g = pc["tab8"].astype(np.float32).reshape(P, plan["nchk"], 128)
        pair_T = pc["pairT"].astype(np.float32)
        pagg = np.zeros((4, P, 256), np.float32)
        for s in plan["segs"]:
            G = g[:, s["chunk"], :]
            A = pc["amat"].astype(np.float32)[:, s["acol"]: s["acol"] + s["hi"] - s["lo"]]
            pagg[s["tile"]][:, s["lo"] - s["tile"] * 256: s["hi"] - s["tile"] * 256] += G.T @ A
        neigh_rawT = np.concatenate(list(pagg), axis=1)
        Wva = cmat[:, S_WVA * 128:(S_WVA + 1) * 128]
        Wvf = cmat[:, S_WVF * 128:(S_WVF + 1) * 128]
        Wk = cmat[:, S_WK * 128:(S_WK + 1) * 128]
        Wq = cmat[:, S_WQ * 128:(S_WQ + 1) * 128]
        actn = Wva.T @ neigh_rawT                     # [128, 1024]
        acts = Wva.T @ pair_T[:, 0:256]               # [128, 256] self
        vf = Wvf.T @ pair_T[:, 256:512]
        kt = Wk.T @ pair_T
        qt = Wq.T @ pair_T
        n2 = (actn * actn).sum(0)
        s2 = (acts * acts).sum(0)
        w_num = cmat[:, S_MUHI * 128:S_MUHI * 128 + 1]
        w_smu = cmat[:, S_MULO * 128:S_MULO * 128 + 1]
        nmu = (w_num * neigh_rawT).sum(0)
        smu = (w_smu * pair_T[:, 0:256]).sum(0)
        den2 = n2 + np.repeat(s2, MC)
        numv = nmu + np.repeat(smu, MC)
        logit = numv / np.sqrt(den2)
        e = np.exp(logit).reshape(BC, MC)
        coef = e / e.sum(1, keepdims=True)
        neighT = actn.reshape(P, BC, MC)
        nsum = (neighT * coef[None]).sum(-1)
        vmid = 0.5 * (acts + nsum)
        saa = (kt[:, 0:256] * qt[:, 0:256]).sum(0) / DOUT
        saf = (kt[:, 0:256] * qt[:, 256:512]).sum(0) / DOUT
        sfa = (kt[:, 256:512] * qt[:, 0:256]).sum(0) / DOUT
        sff = (kt[:, 256:512] * qt[:, 256:512]).sum(0) / DOUT
        waa = 1.0 / (1.0 + np.exp(-(saa - saf)))
        wfa = 1.0 / (1.0 + np.exp(-(sfa - sff)))
        dd = vmid - vf
        new0 = vf + waa[None] * dd
        new1 = vf + wfa[None] * dd
        pre0 = RES_RATE * vmid + (1 - RES_RATE) * new0
        pre1 = RES_RATE * vf + (1 - RES_RATE) * new1
        elu = lambda x: np.where(x > 0, x, np.exp(np.minimum(x, 0)) - 1)
        outs_a.append(elu(pre0).T)
        outs_f.append(elu(pre1).T)
    return np.concatenate(outs_a, 0), np.concatenate(outs_f, 0)


# --------------------------------------------------------------------------
# public entry point
# --------------------------------------------------------------------------

_module_cache = {}
_last_results = None


def _plan_signature(plan):
    return (plan["nchk"], plan["aw"], plan["pieces"],
            tuple((s["chunk"], s["tile"], s["lo"], s["hi"], s["acol"])
                  for s in plan["segs"]))


def kernel(**inputs):
    plan, percore = preprocess(inputs)
    sig = _plan_signature(plan)
    if sig not in _module_cache:
        _module_cache[sig] = build_module(plan)
    nc = _module_cache[sig]

    cmat = make_consts(inputs)
    in_maps = []
    for c in range(NCORES):
        pc = percore[c]
        pk1 = np.concatenate([cmat, pc["pairT"]], axis=1)
        in_maps.append({
            "pk1": np.ascontiguousarray(pk1),
            "amat": pc["amat"],
            "tab8": pc["tab8"],
        })
    res = run_bass_kernel_spmd(nc, in_maps, core_ids=list(range(NCORES)))
    global _last_results
    _last_results = res
    # out[d, h, c, b]: node h*128+b of this core, branch c (0=agg, 1=ff)
    aggs, ffs = [], []
    for c in range(NCORES):
        o = res.results[c]["out"].astype(np.float32)   # [128, 2, 2, 128]
        aggs.append(o[:, :, 0, :].reshape(P, BC).T)
        ffs.append(o[:, :, 1, :].reshape(P, BC).T)
    return np.concatenate(aggs, 0), np.concatenate(ffs, 0)
